# revision 7
# baseline (speedup 1.0000x reference)
"""OFT block-diagonal rotation forward (nn_Linear_12635793785535).

y = x @ blockdiag(rot_0..rot_63), rot_r = I + 2Q_r + 2Q_r^2 + 2Q_r^3 + 2Q_r^4
with Q_r the skew-symmetric matrix built from weight[r].

Sharding: data-parallel over tokens across 8 NeuronCores; the small derived
rotation blocks are replicated (per the problem's sharding hint).

Device kernel per core (1024 tokens, 4096 features):
  for each 128-token tile:
    DMA x tile [128, 4096] (natural layout, contiguous)
    for each group of 4 feature-pairs (512 features):
      4x PE transpose x chunks -> one PSUM bank [128, 512]; 1 DVE copy -> SBUF
      4x PE matmul out[tok, feat] = xT.T @ rotpair -> one PSUM bank [128, 512]
      1 copy PSUM -> y SBUF tile (alternating DVE / ACT)
    DMA y tile out
Transposes run as float32r (bit-preserving data movement, 1.5 vs 2 cyc/row).
"""

import numpy as np

TOKENS = 8192
FEAT = 4096
R = 64
BLOCK = 64
NPAIR = 32  # pairs of 64-blocks -> 128-wide block-diagonal tiles
GROUP = 4  # pairs per PSUM bank group (4 x 128 = 512 wide)
NGROUP = NPAIR // GROUP  # 8
NUM_TERMS = 5
N_CORES = 8
TOK_SHARD = TOKENS // N_CORES  # 1024
TOK_TILE = 128
N_TTILES = TOK_SHARD // TOK_TILE  # 8

F32R_TRANSPOSE = False

_CACHE = {}

# test.py can flip these before calling kernel()
TRACE = False
LAST_RESULTS = None


def _build_bass():
    from contextlib import ExitStack

    import concourse.tile as tile
    from concourse import bacc, mybir
    from concourse.masks import make_identity

    nc = bacc.Bacc(
        "TRN2",
        target_bir_lowering=False,
        debug=False,
        enable_asserts=False,
        num_devices=N_CORES,
    )
    x_d = nc.dram_tensor(
        "x", [TOK_SHARD, FEAT], mybir.dt.float32, kind="ExternalInput"
    ).ap()
    # rot layout [k=128, pair, c=128]: per-partition contiguous rows for DMA
    rot_d = nc.dram_tensor(
        "rot", [128, NPAIR, 128], mybir.dt.float32, kind="ExternalInput"
    ).ap()
    y_d = nc.dram_tensor(
        "y", [TOK_SHARD, FEAT], mybir.dt.float32, kind="ExternalOutput"
    ).ap()

    with tile.TileContext(nc) as tc, ExitStack() as ctx:
        const_pool = ctx.enter_context(tc.tile_pool(name="const", bufs=1))
        xpool = ctx.enter_context(tc.tile_pool(name="xin", bufs=3))
        ypool = ctx.enter_context(tc.tile_pool(name="yout", bufs=3))
        xtpool = ctx.enter_context(tc.tile_pool(name="xt", bufs=4))
        ps_t = ctx.enter_context(tc.tile_pool(name="ps_t", bufs=3, space="PSUM"))
        ps_y = ctx.enter_context(tc.tile_pool(name="ps_y", bufs=3, space="PSUM"))

        ident = const_pool.tile([128, 128], mybir.dt.float32)
        make_identity(nc, ident)

        rot_sb = const_pool.tile([128, NPAIR, 128], mybir.dt.float32)
        nc.sync.dma_start(rot_sb[:], rot_d)

        SLAB = 1024  # feature columns per DMA slab (2 groups per slab)
        NSLAB = FEAT // SLAB  # 4
        for t in range(N_TTILES):
            tok = slice(t * TOK_TILE, (t + 1) * TOK_TILE)
            # independent slab tiles so compute starts after the first slab
            x_slabs = []
            for s in range(NSLAB):
                xsl = xpool.tile([TOK_TILE, SLAB], mybir.dt.float32, name="xsl", tag="xsl")
                nc.sync.dma_start(xsl[:], x_d[tok, s * SLAB : (s + 1) * SLAB])
                x_slabs.append(xsl)
            y_slabs = [
                ypool.tile([TOK_TILE, SLAB], mybir.dt.float32, name="ysl", tag="ysl")
                for _ in range(NSLAB)
            ]
            for g in range(NGROUP):
                s = g // 2  # slab index; 2 groups per slab
                gc = (g % 2) * GROUP * 128  # column offset within slab
                xt_ps = ps_t.tile([128, GROUP * TOK_TILE], mybir.dt.float32)
                for j in range(GROUP):
                    src = x_slabs[s][:, gc + j * 128 : gc + (j + 1) * 128]
                    dst = xt_ps[:, j * TOK_TILE : (j + 1) * TOK_TILE]
                    nc.tensor.transpose(dst, src, ident[:])
                xt_sb = xtpool.tile([128, GROUP * TOK_TILE], mybir.dt.float32)
                nc.vector.tensor_copy(xt_sb[:], xt_ps[:])
                y_ps = ps_y.tile([TOK_TILE, GROUP * 128], mybir.dt.float32)
                for j in range(GROUP):
                    p = g * GROUP + j
                    nc.tensor.matmul(
                        y_ps[:, j * 128 : (j + 1) * 128],
                        xt_sb[:, j * TOK_TILE : (j + 1) * TOK_TILE],
                        rot_sb[:, p, :],
                        start=True,
                        stop=True,
                    )
                # ACT takes all y copies; DVE keeps the xT copies
                nc.scalar.copy(y_slabs[s][:, gc : gc + GROUP * 128], y_ps[:])
                if g % 2 == 1:
                    # slab complete -> drain it while later groups compute
                    nc.scalar.dma_start(
                        y_d[tok, s * SLAB : (s + 1) * SLAB], y_slabs[s][:]
                    )

    nc.compile()
    return nc


def _host_rot_layout(weight):
    """Cayley-Neumann series on host (f32), packed as [k=128, pair, c=128]
    block-diagonal pair tiles (replicated across cores per sharding hint)."""
    w = np.asarray(weight, dtype=np.float32)
    rows, cols = np.triu_indices(BLOCK, k=1)
    Q = np.zeros((R, BLOCK, BLOCK), dtype=np.float32)
    Q[:, rows, cols] = w
    Q = Q - np.swapaxes(Q, 1, 2)
    eye = np.eye(BLOCK, dtype=np.float32)
    rot = eye[None, :, :] + 2.0 * Q
    Qp = Q
    for _ in range(2, NUM_TERMS):
        Qp = np.einsum("rij,rjk->rik", Qp, Q).astype(np.float32)
        rot = rot + 2.0 * Qp
    layout = np.zeros((128, NPAIR, 128), dtype=np.float32)
    for pair in range(NPAIR):
        layout[0:64, pair, 0:64] = rot[2 * pair]
        layout[64:128, pair, 64:128] = rot[2 * pair + 1]
    return layout


def kernel(x, weight):
    global LAST_RESULTS
    if "nc" not in _CACHE:
        _CACHE["nc"] = _build_bass()
    nc = _CACHE["nc"]

    from concourse.bass_utils import run_bass_kernel_spmd

    x = np.ascontiguousarray(np.asarray(x, dtype=np.float32))
    rot = _host_rot_layout(weight)
    in_maps = [
        {
            "x": np.ascontiguousarray(x[i * TOK_SHARD : (i + 1) * TOK_SHARD]),
            "rot": rot,
        }
        for i in range(N_CORES)
    ]
    res = run_bass_kernel_spmd(
        nc, in_maps, core_ids=list(range(N_CORES)), trace=TRACE
    )
    LAST_RESULTS = res
    out = np.concatenate([r["y"] for r in res.results], axis=0)
    return out


# revision 8
# speedup vs baseline: 1.1154x; 1.1154x over previous
"""OFT block-diagonal rotation forward (nn_Linear_12635793785535).

y = x @ blockdiag(rot_0..rot_63), rot_r = I + 2Q_r + 2Q_r^2 + 2Q_r^3 + 2Q_r^4
with Q_r the skew-symmetric matrix built from weight[r].

Sharding: data-parallel over tokens across 8 NeuronCores; the small derived
rotation blocks are replicated (per the problem's sharding hint).

Device kernel per core (1024 tokens, 4096 features):
  for each 128-token tile:
    DMA x tile [128, 4096] (natural layout, contiguous)
    for each group of 4 feature-pairs (512 features):
      4x PE transpose x chunks -> one PSUM bank [128, 512]; 1 DVE copy -> SBUF
      4x PE matmul out[tok, feat] = xT.T @ rotpair -> one PSUM bank [128, 512]
      1 copy PSUM -> y SBUF tile (alternating DVE / ACT)
    DMA y tile out
Transposes run as float32r (bit-preserving data movement, 1.5 vs 2 cyc/row).
"""

import numpy as np

TOKENS = 8192
FEAT = 4096
R = 64
BLOCK = 64
NPAIR = 32  # pairs of 64-blocks -> 128-wide block-diagonal tiles
GROUP = 4  # pairs per PSUM bank group (4 x 128 = 512 wide)
NGROUP = NPAIR // GROUP  # 8
NUM_TERMS = 5
N_CORES = 8
TOK_SHARD = TOKENS // N_CORES  # 1024
TOK_TILE = 128
N_TTILES = TOK_SHARD // TOK_TILE  # 8

F32R_TRANSPOSE = False

_CACHE = {}

# test.py can flip these before calling kernel()
TRACE = False
LAST_RESULTS = None


def _build_bass():
    from contextlib import ExitStack

    import concourse.tile as tile
    from concourse import bacc, mybir
    from concourse.masks import make_identity

    nc = bacc.Bacc(
        "TRN2",
        target_bir_lowering=False,
        debug=False,
        enable_asserts=False,
        num_devices=N_CORES,
    )
    x_d = nc.dram_tensor(
        "x", [TOK_SHARD, FEAT], mybir.dt.float32, kind="ExternalInput"
    ).ap()
    # rot layout [k=128, pair, c=128]: per-partition contiguous rows for DMA
    rot_d = nc.dram_tensor(
        "rot", [128, NPAIR, 128], mybir.dt.float32, kind="ExternalInput"
    ).ap()
    y_d = nc.dram_tensor(
        "y", [TOK_SHARD, FEAT], mybir.dt.float32, kind="ExternalOutput"
    ).ap()

    with tile.TileContext(nc) as tc, ExitStack() as ctx:
        const_pool = ctx.enter_context(tc.tile_pool(name="const", bufs=1))
        xpool = ctx.enter_context(tc.tile_pool(name="xin", bufs=3))
        ypool = ctx.enter_context(tc.tile_pool(name="yout", bufs=3))
        xtpool = ctx.enter_context(tc.tile_pool(name="xt", bufs=4))
        ps_t = ctx.enter_context(tc.tile_pool(name="ps_t", bufs=3, space="PSUM"))
        ps_y = ctx.enter_context(tc.tile_pool(name="ps_y", bufs=3, space="PSUM"))

        ident = const_pool.tile([128, 128], mybir.dt.float32)
        make_identity(nc, ident)

        rot_sb = const_pool.tile([128, NPAIR, 128], mybir.dt.float32)
        nc.sync.dma_start(rot_sb[:], rot_d)

        SLAB = 1024  # feature columns per DMA slab (2 groups per slab)
        NSLAB = FEAT // SLAB  # 4
        for t in range(N_TTILES):
            tok = slice(t * TOK_TILE, (t + 1) * TOK_TILE)
            # independent slab tiles so compute starts after the first slab
            x_slabs = []
            for s in range(NSLAB):
                xsl = xpool.tile([TOK_TILE, SLAB], mybir.dt.float32, name="xsl", tag="xsl", bufs=6)
                nc.sync.dma_start(xsl[:], x_d[tok, s * SLAB : (s + 1) * SLAB])
                x_slabs.append(xsl)
            y_slabs = [
                ypool.tile([TOK_TILE, SLAB], mybir.dt.float32, name="ysl", tag="ysl", bufs=6)
                for _ in range(NSLAB)
            ]
            for g in range(NGROUP):
                s = g // 2  # slab index; 2 groups per slab
                gc = (g % 2) * GROUP * 128  # column offset within slab
                xt_ps = ps_t.tile([128, GROUP * TOK_TILE], mybir.dt.float32)
                for j in range(GROUP):
                    src = x_slabs[s][:, gc + j * 128 : gc + (j + 1) * 128]
                    dst = xt_ps[:, j * TOK_TILE : (j + 1) * TOK_TILE]
                    nc.tensor.transpose(dst, src, ident[:])
                xt_sb = xtpool.tile([128, GROUP * TOK_TILE], mybir.dt.float32)
                nc.vector.tensor_copy(xt_sb[:], xt_ps[:])
                y_ps = ps_y.tile([TOK_TILE, GROUP * 128], mybir.dt.float32)
                for j in range(GROUP):
                    p = g * GROUP + j
                    nc.tensor.matmul(
                        y_ps[:, j * 128 : (j + 1) * 128],
                        xt_sb[:, j * TOK_TILE : (j + 1) * TOK_TILE],
                        rot_sb[:, p, :],
                        start=True,
                        stop=True,
                    )
                # ACT takes all y copies; DVE keeps the xT copies
                nc.scalar.copy(y_slabs[s][:, gc : gc + GROUP * 128], y_ps[:])
                if g % 2 == 1:
                    # slab complete -> drain it while later groups compute
                    nc.scalar.dma_start(
                        y_d[tok, s * SLAB : (s + 1) * SLAB], y_slabs[s][:]
                    )

    nc.compile()
    return nc


def _host_rot_layout(weight):
    """Cayley-Neumann series on host (f32), packed as [k=128, pair, c=128]
    block-diagonal pair tiles (replicated across cores per sharding hint)."""
    w = np.asarray(weight, dtype=np.float32)
    rows, cols = np.triu_indices(BLOCK, k=1)
    Q = np.zeros((R, BLOCK, BLOCK), dtype=np.float32)
    Q[:, rows, cols] = w
    Q = Q - np.swapaxes(Q, 1, 2)
    eye = np.eye(BLOCK, dtype=np.float32)
    rot = eye[None, :, :] + 2.0 * Q
    Qp = Q
    for _ in range(2, NUM_TERMS):
        Qp = np.einsum("rij,rjk->rik", Qp, Q).astype(np.float32)
        rot = rot + 2.0 * Qp
    layout = np.zeros((128, NPAIR, 128), dtype=np.float32)
    for pair in range(NPAIR):
        layout[0:64, pair, 0:64] = rot[2 * pair]
        layout[64:128, pair, 64:128] = rot[2 * pair + 1]
    return layout


def kernel(x, weight):
    global LAST_RESULTS
    if "nc" not in _CACHE:
        _CACHE["nc"] = _build_bass()
    nc = _CACHE["nc"]

    from concourse.bass_utils import run_bass_kernel_spmd

    x = np.ascontiguousarray(np.asarray(x, dtype=np.float32))
    rot = _host_rot_layout(weight)
    in_maps = [
        {
            "x": np.ascontiguousarray(x[i * TOK_SHARD : (i + 1) * TOK_SHARD]),
            "rot": rot,
        }
        for i in range(N_CORES)
    ]
    res = run_bass_kernel_spmd(
        nc, in_maps, core_ids=list(range(N_CORES)), trace=TRACE
    )
    LAST_RESULTS = res
    out = np.concatenate([r["y"] for r in res.results], axis=0)
    return out
